# revision 3
# baseline (speedup 1.0000x reference)
"""Trainium2 Bass kernel for cross-attention:
    scores  = dec @ enc^T            [B, Tq, Tk]
    probs   = softmax(scores, -1)
    context = probs @ enc            [B, Tq, D]

Shapes (hardcoded): enc [16, 2048, 1024] f32, dec [16, 128, 1024] f32.
Sharding: data-parallel over batch B across 8 NeuronCores (2 batches/core).

Per-core per-batch flow:
  - dec loaded, PE-transposed to decT [d, q] (fp32r, exact-ish rounding).
  - enc streamed in 4 chunks of 512 k-rows; each chunk's 8 d-tiles are
    PE-transposed (transpose-mode matmul) into PSUM, copied to SBUF
    (DVE/ACT alternating), then mm1 (fp32r, N=512) accumulates
    scores [q=128, k=512] per chunk; per-chunk max reduced as it lands.
  - softmax: combine chunk maxes (negated) -> ACT exp with bias and
    accum_out partial sums -> reduce_sum + reciprocal on DVE.
  - probs PE-transposed to probsT [k, q]; mm2 (fp32r) with rhs = the
    resident natural enc tiles, probsT weights reused across both
    d-halves; ACT Copy with per-partition 1/denom scale evicts to SBUF.
Transposes are interleaved with matmuls in PE program order so real
matmul activity keeps the HAM clock gate at full rate.
fp32r keeps ~13 effective mantissa bits (measured ~1.5e-4) — output
rel err vs fp32 reference ~8e-4.
"""

import sys

sys.path.insert(0, "/opt/trn_rl_repo")

import numpy as np
from contextlib import ExitStack

import concourse.bass as bass
import concourse.tile as tile
from concourse import bacc, mybir
from concourse.masks import make_identity

F32 = mybir.dt.float32
F32R = mybir.dt.float32r
EXP = mybir.ActivationFunctionType.Exp
COPY = mybir.ActivationFunctionType.Copy
AX_X = mybir.AxisListType.X

B, Tk, Tq, D = 16, 2048, 128, 1024
CORES = 8
BLOC = B // CORES          # batches per core
KCH = 4                    # k chunks per batch
KCS = Tk // KCH            # 512 k rows per chunk
NSUB = KCS // 128          # 4 k-subtiles per chunk
DT = D // 128              # 8 d-tiles
DH = D // 512              # 2 output column halves

_CACHE = {}


def _build():
    nc = bacc.Bacc("TRN2", debug=False, num_devices=CORES)
    enc = nc.dram_tensor("enc", [BLOC, Tk, D], F32, kind="ExternalInput").ap()
    dec = nc.dram_tensor("dec", [BLOC, Tq, D], F32, kind="ExternalInput").ap()
    out = nc.dram_tensor("out", [BLOC, Tq, D], F32, kind="ExternalOutput").ap()

    with tile.TileContext(nc) as tc, ExitStack() as ctx:
        sb = ctx.enter_context(tc.tile_pool(name="sb", bufs=1))
        enc_p = ctx.enter_context(tc.tile_pool(name="enc", bufs=7))
        encT_p = ctx.enter_context(tc.tile_pool(name="encT", bufs=10))
        dec_p = ctx.enter_context(tc.tile_pool(name="dec", bufs=2))
        decT_p = ctx.enter_context(tc.tile_pool(name="decT", bufs=4))
        probs_p = ctx.enter_context(tc.tile_pool(name="probs", bufs=1))
        probsT_p = ctx.enter_context(tc.tile_pool(name="probsT", bufs=8))
        outp_p = ctx.enter_context(tc.tile_pool(name="outp", bufs=2))
        stat_p = ctx.enter_context(tc.tile_pool(name="stat", bufs=4))
        sc_p = ctx.enter_context(tc.tile_pool(name="sc", bufs=1, space="PSUM"))
        tr_p = ctx.enter_context(tc.tile_pool(name="tr", bufs=2, space="PSUM"))
        ctx_p = ctx.enter_context(tc.tile_pool(name="ctx", bufs=2, space="PSUM"))

        ident = sb.tile([128, 128], F32)
        ident_r = sb.tile([128, 128], F32R)
        make_identity(nc, ident[:])
        nc.vector.tensor_copy(ident_r[:], ident[:])

        def psum2sbuf(dst, src, engine):
            if engine == "dve":
                nc.vector.tensor_copy(dst, src)
            else:
                nc.scalar.copy(dst, src)

        for b in range(BLOC):
            # ---- dec load + transpose -> decT blocks [d, q] ----
            dec_sb = dec_p.tile([128, D], F32R, tag="dec")
            nc.sync.dma_start(dec_sb[:], dec[b].bitcast(F32R))
            decT = []
            for blk in range(2):
                trt = tr_p.tile([128, 512], F32R, tag="tr")
                for j in range(4):
                    d = 4 * blk + j
                    nc.tensor.transpose(
                        trt[:, 128 * j : 128 * (j + 1)],
                        dec_sb[:, 128 * d : 128 * (d + 1)],
                        ident_r[:],
                    )
                dstT = decT_p.tile([128, 512], F32R, tag="decT")
                psum2sbuf(dstT[:], trt[:], "dve" if blk == 0 else "act")
                decT.append(dstT)

            # ---- enc chunks: load, transpose, mm1 into scores psum ----
            # PE program order interleaves transpose groups with mm1
            # matmuls (the real matmuls keep the HAM clock warm).
            scores = sc_p.tile([128, Tk], F32, tag="sc")
            maxes = stat_p.tile([128, KCH], F32, tag="maxes")
            enc_sb = []
            pending_mm1 = []

            def emit_mm1(kc, d, encT_d):
                nc.tensor.matmul(
                    scores[:, kc * KCS : (kc + 1) * KCS],
                    decT[d // 4][:, 128 * (d % 4) : 128 * (d % 4 + 1)],
                    encT_d[:],
                    start=(d == 0),
                    stop=(d == DT - 1),
                )

            for kc in range(KCH):
                et = enc_p.tile([128, NSUB, D], F32R, tag="enc")
                for n in range(NSUB):
                    nc.sync.dma_start(
                        et[:, n, :],
                        enc[b, kc * KCS + 128 * n : kc * KCS + 128 * (n + 1), :]
                        .rearrange("p d -> p d")
                        .bitcast(F32R),
                    )
                enc_sb.append(et)
                for d in range(DT):
                    trt = tr_p.tile([128, 512], F32R, tag="tr")
                    for n in range(NSUB):
                        nc.tensor.transpose(
                            trt[:, 128 * n : 128 * (n + 1)],
                            et[:, n, 128 * d : 128 * (d + 1)],
                            ident_r[:],
                        )
                    eT = encT_p.tile([128, 512], F32R, tag="encT")
                    psum2sbuf(eT[:], trt[:], "dve" if d % 2 == 0 else "act")
                    pending_mm1.append((kc, d, eT))
                    # stay one group behind the transposes so the copy
                    # has landed by the time the matmul wants its rhs
                    if len(pending_mm1) >= 2:
                        emit_mm1(*pending_mm1.pop(0))
                # per-chunk running max (DVE) as soon as the chunk is done
                if len(pending_mm1) == 1 and pending_mm1[0][1] == DT - 1:
                    emit_mm1(*pending_mm1.pop(0))
                nc.vector.reduce_max(
                    maxes[:, kc : kc + 1],
                    scores[:, kc * KCS : (kc + 1) * KCS],
                    axis=AX_X,
                )
            while pending_mm1:
                emit_mm1(*pending_mm1.pop(0))

            # ---- softmax ----
            negmax = stat_p.tile([128, 1], F32, tag="negmax")
            nc.vector.reduce_max(negmax[:], maxes[:], axis=AX_X, negate=True)
            probs = probs_p.tile([128, Tk], F32, tag="probs")
            sums = stat_p.tile([128, KCH], F32, tag="sums")
            for kc in range(KCH):
                nc.scalar.activation(
                    probs[:, kc * KCS : (kc + 1) * KCS],
                    scores[:, kc * KCS : (kc + 1) * KCS],
                    EXP,
                    bias=negmax[:],
                    scale=1.0,
                    accum_out=sums[:, kc : kc + 1],
                )
            denom = stat_p.tile([128, 1], F32, tag="denom")
            nc.vector.reduce_sum(denom[:], sums[:], axis=AX_X)
            rdenom = stat_p.tile([128, 1], F32, tag="rdenom")
            nc.vector.reciprocal(rdenom[:], denom[:])

            # ---- probs transpose -> probsT blocks [k, q] ----
            probsT = []
            for blk in range(4):
                trt = tr_p.tile([128, 512], F32, tag="tr")
                for j in range(4):
                    t = 4 * blk + j
                    nc.tensor.transpose(
                        trt[:, 128 * j : 128 * (j + 1)],
                        probs[:, 128 * t : 128 * (t + 1)],
                        ident[:],
                    )
                pT = probsT_p.tile([128, 512], F32R, tag="probsT")
                psum2sbuf(pT[:], trt[:], "dve" if blk % 2 == 0 else "act")
                probsT.append(pT)

            # ---- mm2 (dh-inner: probsT weights reused) + normalize ----
            out_sb = outp_p.tile([128, D], F32, tag="outp")
            cps = [
                ctx_p.tile([128, 512], F32, tag="ctx", name=f"cps{b}_{dh}")
                for dh in range(DH)
            ]
            for t in range(4 * KCH):
                for dh in range(DH):
                    nc.tensor.matmul(
                        cps[dh][:],
                        probsT[t // 4][:, 128 * (t % 4) : 128 * (t % 4 + 1)],
                        enc_sb[t // 4][:, t % 4, dh * 512 : (dh + 1) * 512],
                        start=(t == 0),
                        stop=(t == 4 * KCH - 1),
                    )
            for dh in range(DH):
                nc.scalar.activation(
                    out_sb[:, dh * 512 : (dh + 1) * 512],
                    cps[dh][:],
                    COPY,
                    bias=0.0,
                    scale=rdenom[:],
                )
            nc.sync.dma_start(out[b], out_sb[:])

    nc.compile()
    return nc


def kernel(encoder_hiddens: np.ndarray, decoder_hidden: np.ndarray) -> np.ndarray:
    enc = np.ascontiguousarray(np.asarray(encoder_hiddens, dtype=np.float32))
    dec = np.ascontiguousarray(np.asarray(decoder_hidden, dtype=np.float32))
    assert enc.shape == (B, Tk, D) and dec.shape == (B, Tq, D)

    if "nc" not in _CACHE:
        _CACHE["nc"] = _build()
    nc = _CACHE["nc"]

    from concourse.bass_utils import run_bass_kernel_spmd

    in_maps = [
        {
            "enc": enc[c * BLOC : (c + 1) * BLOC],
            "dec": dec[c * BLOC : (c + 1) * BLOC],
        }
        for c in range(CORES)
    ]
    res = run_bass_kernel_spmd(nc, in_maps, core_ids=list(range(CORES)))
    out = np.empty((B, Tq, D), dtype=np.float32)
    for c in range(CORES):
        out[c * BLOC : (c + 1) * BLOC] = res.results[c]["out"]
    return out
